# revision 1
# baseline (speedup 1.0000x reference)
"""Trainium2 Bass kernel for the AttentionBlock problem.

Reference semantics (shapes hardcoded):
    x [4, 256, 64, 64]; 1x1-conv weights q_w/k_w/v_w [256, 258] (+biases),
    fc_w [256, 256], fc_b [256].
    x0 = concat(x, pos) -> [B, 258, 4096]
    q/k/v = relu(W @ x0 + b)                    [B, 256, 4096]
    attn  = softmax_causal(q^T k)               [B, 4096, 4096]
    out   = x + relu(fc_w @ (attn @ v^T)^T + fc_b)

Distribution: 8 cores = 4 batches x 2 query-block roles. Each core
computes full k / v^T for its batch, q only for its 4 owned 512-wide
query blocks, and causal attention for those blocks. Causal work is
balanced by giving role 0 global blocks [0,3,4,7] and role 1 blocks
[1,2,5,6]; both roles run the identical SPMD program with per-slot
key-tile counts [8,16,24,32] (slightly padded), with per-core mask
data zeroing padded/non-causal entries.

Softmax is computed without max-subtraction (scores are ~26+-5, far
from fp32 overflow): p = exp(s) * mask, normalized by a replicated
ones-matmul denominator (the [128,128] ones lhsT yields the column
sums broadcast across all partitions, so no separate broadcast step).

Precision split: the score path (q/k projections, q^T k) runs in
float32r (~16-bit-mantissa fp32 matmul mode at full PE speed; exp
amplifies absolute score error so bf16 is not enough there).
Everything whose error enters linearly (exp(p) weights, v, fc, masks)
runs in bf16. PE dtype switches (f32r <-> bf16) cost ~100-200ns, so
matmuls are emitted in same-dtype batches; dtypes are never mixed
within one PSUM accumulation group (mixed groups crash the HW).

The positional-embedding + bias contribution of each projection is a
batch-independent rank-3 map, precomputed on the host and added via
VectorE between the PSUM matmul result and the ScalarE relu.
"""

import numpy as np

B = 4
C = 256
S = 64
N = S * S            # 4096
K = 256              # q/k/v channels
NBLK = 512           # query block width
NSLOT = 4            # owned query blocks per core
M_S = (8, 16, 24, 32)  # key-tile count per slot (128-wide key tiles)
BLOCKS = ((0, 3, 4, 7), (1, 2, 5, 6))  # role -> global block ids

_PROGRAM = None


def _build_program():
    import concourse.bacc as bacc
    import concourse.mybir as mybir
    import concourse.tile as tile

    F32 = mybir.dt.float32
    F32R = mybir.dt.float32r
    BF16 = mybir.dt.bfloat16
    Act = mybir.ActivationFunctionType

    nc = bacc.Bacc("TRN2", target_bir_lowering=False, debug=False)

    x0f_d = nc.dram_tensor("x0f", [C, N], F32R, kind="ExternalInput")
    x0b_d = nc.dram_tensor("x0b", [C, N], BF16, kind="ExternalInput")
    xqf_d = nc.dram_tensor("xqf", [C, NSLOT * NBLK], F32R, kind="ExternalInput")
    wqf_d = nc.dram_tensor("wqf", [C, K], F32R, kind="ExternalInput")
    wkf_d = nc.dram_tensor("wkf", [C, K], F32R, kind="ExternalInput")
    wv_d = nc.dram_tensor("wv", [C, K], BF16, kind="ExternalInput")
    x0p_d = nc.dram_tensor("x0p", [128, N], F32R, kind="ExternalInput")
    x0c_d = nc.dram_tensor("x0c", [3, N], BF16, kind="ExternalInput")
    xqp_d = nc.dram_tensor("xqp", [128, NSLOT * NBLK], F32R,
                           kind="ExternalInput")
    wqp_d = nc.dram_tensor("wqp", [128, K], F32R, kind="ExternalInput")
    wkp_d = nc.dram_tensor("wkp", [128, K], F32R, kind="ExternalInput")
    wvc_d = nc.dram_tensor("wvc", [3, K], BF16, kind="ExternalInput")
    fcw_d = nc.dram_tensor("fcw", [C, C], BF16, kind="ExternalInput")
    fcb_d = nc.dram_tensor("fcb", [C, 1], F32, kind="ExternalInput")
    msk_d = nc.dram_tensor("masks", [NSLOT, 8, 128, NBLK], BF16,
                           kind="ExternalInput")
    ob_d = nc.dram_tensor("ones_b", [128, 128], BF16, kind="ExternalInput")
    out_d = nc.dram_tensor("out", [C, NSLOT * NBLK], F32, kind="ExternalOutput")

    with tile.TileContext(nc) as tc:
        with (
            tc.tile_pool(name="wts", bufs=1) as wts,
            tc.tile_pool(name="xq_p", bufs=1) as xq_p,
            tc.tile_pool(name="x0_p", bufs=3) as x0_p,
            tc.tile_pool(name="kqv_p", bufs=1) as kqv_p,
            tc.tile_pool(name="msk_p", bufs=6) as msk_p,
            tc.tile_pool(name="ex_p", bufs=9) as ex_p,
            tc.tile_pool(name="ds_p", bufs=3) as ds_p,
            tc.tile_pool(name="o_p", bufs=4) as o_p,
            tc.tile_pool(name="rb_p", bufs=2) as rb_p,
            tc.tile_pool(name="tr_p", bufs=3) as tr_p,
            tc.tile_pool(name="ps_sc", bufs=4, space="PSUM") as ps_sc,
            tc.tile_pool(name="ps_out", bufs=1, space="PSUM") as ps_out,
            tc.tile_pool(name="ps_den", bufs=1, space="PSUM") as ps_den,
            tc.tile_pool(name="ps_fc", bufs=1, space="PSUM") as ps_fc,
        ):
            def wtile(dram, r0, rn, dt, tag):
                t = wts.tile([rn, dram.shape[1]], dt, tag=tag, name=tag)
                nc.sync.dma_start(t[:], dram[r0:r0 + rn, :])
                return t

            # weights needed by phase A first (k, v) so PE can start early
            wk_t = [wtile(wkf_d, 0, 128, F32R, "wk0"),
                    wtile(wkf_d, 128, 128, F32R, "wk1"),
                    wtile(wkp_d, 0, 128, F32R, "wk2")]
            wv_t = [wtile(wv_d, 0, 128, BF16, "wv0"),
                    wtile(wv_d, 128, 128, BF16, "wv1"),
                    wtile(wvc_d, 0, 3, BF16, "wv2")]

            # ---- phase A: k and vT per position-block pair; pair s is
            # emitted just before attention slot s (its k tiles are the
            # last 8 m-tiles that slot consumes) ----
            k_sb = [[None] * 8 for _ in range(2)]
            vT_sb = [None] * 32

            def emit_pair(nbp):
                xfp, xbp = [], []
                for nb in (2 * nbp, 2 * nbp + 1):
                    sl = slice(NBLK * nb, NBLK * (nb + 1))
                    xf = []
                    for ci, (dram, r0) in enumerate(
                            ((x0f_d, 0), (x0f_d, 128), (x0p_d, 0))):
                        t = x0_p.tile([128, NBLK], F32R, tag=f"xf{ci}",
                                      name=f"xf{ci}_{nb}")
                        nc.sync.dma_start(t[:], dram[r0:r0 + 128, sl])
                        xf.append(t)
                    xfp.append(xf)
                    xb = []
                    for ci, (dram, r0, rn) in enumerate(
                            ((x0b_d, 0, 128), (x0b_d, 128, 128),
                             (x0c_d, 0, 3))):
                        tb = x0_p.tile([rn, NBLK], BF16, tag=f"xb{ci}",
                                       name=f"xb{ci}_{nb}")
                        nc.sync.dma_start(tb[:], dram[r0:r0 + rn, sl])
                        xb.append(tb)
                    xbp.append(xb)

                for li, nb in enumerate((2 * nbp, 2 * nbp + 1)):
                    for kt in range(2):
                        kts = slice(128 * kt, 128 * (kt + 1))
                        pk = ps_sc.tile([128, NBLK], F32, tag="sc",
                                        name=f"pk{kt}_{nb}")
                        for ci in range(3):
                            nc.tensor.matmul(pk[:], wk_t[ci][:, kts],
                                             xfp[li][ci][:],
                                             start=(ci == 0), stop=(ci == 2))
                        kt_sb = kqv_p.tile([128, NBLK], F32R,
                                           tag=f"k{kt}_{nb}",
                                           name=f"k{kt}_{nb}")
                        nc.scalar.activation(kt_sb[:], pk[:], Act.Relu)
                        k_sb[kt][nb] = kt_sb
                for li, nb in enumerate((2 * nbp, 2 * nbp + 1)):
                    for sub in range(4):
                        i = 4 * nb + sub
                        ss = slice(128 * sub, 128 * (sub + 1))
                        pv = ps_sc.tile([128, K], F32, tag="sc",
                                        name=f"pv{i}")
                        for ci in range(3):
                            nc.tensor.matmul(pv[:], xbp[li][ci][:, ss],
                                             wv_t[ci][:],
                                             start=(ci == 0), stop=(ci == 2))
                        vt_sb = kqv_p.tile([128, K], BF16, tag=f"v{i}",
                                           name=f"v{i}")
                        nc.scalar.activation(pv_out := vt_sb[:], pv[:],
                                             Act.Relu)
                        vT_sb[i] = vt_sb

            emit_pair(0)

            # secondary inputs: emitted after pair-0 DMAs so they do not
            # delay the first k matmuls, but early enough that q is ready
            # when the pairs finish
            wq_t = [wtile(wqf_d, 0, 128, F32R, "wq0"),
                    wtile(wqf_d, 128, 128, F32R, "wq1"),
                    wtile(wqp_d, 0, 128, F32R, "wq2")]
            fcw_t = [wtile(fcw_d, 0, 128, BF16, "fcw0"),
                     wtile(fcw_d, 128, 128, BF16, "fcw1")]
            fcb_t = [wtile(fcb_d, 0, 128, F32, "fcb0"),
                     wtile(fcb_d, 128, 128, F32, "fcb1")]
            ones_b = wtile(ob_d, 0, 128, BF16, "ones_b")
            xq_t = []
            for ci, (dram, r0) in enumerate(
                    ((xqf_d, 0), (xqf_d, 128), (xqp_d, 0))):
                t = xq_p.tile([128, NSLOT * NBLK], F32R, tag=f"xq{ci}",
                              name=f"xq{ci}")
                nc.sync.dma_start(t[:], dram[r0:r0 + 128, :])
                xq_t.append(t)

            for _nbp in range(1, 4):
                emit_pair(_nbp)

            # ---- phase A part 2: q per slot ----
            q_sb = [[None] * NSLOT for _ in range(2)]
            for s in range(NSLOT):
                sl = slice(NBLK * s, NBLK * (s + 1))
                for kt in range(2):
                    kts = slice(128 * kt, 128 * (kt + 1))
                    pq = ps_sc.tile([128, NBLK], F32, tag="sc",
                                    name=f"pq{kt}_{s}")
                    for ci in range(3):
                        nc.tensor.matmul(pq[:], wq_t[ci][:, kts],
                                         xq_t[ci][:, sl],
                                         start=(ci == 0), stop=(ci == 2))
                    qt = kqv_p.tile([128, NBLK], F32R, tag=f"q{kt}_{s}",
                                    name=f"q{kt}_{s}")
                    nc.scalar.activation(qt[:], pq[:], Act.Relu)
                    q_sb[kt][s] = qt

            # ---- phase B: attention + fc per slot ----
            def finalize_slot(s, po, pd):
                """normalize slot s, fc, relu, residual, dma out."""
                rb_sb = rb_p.tile([128, NBLK], F32, tag="rb", name=f"rb{s}")
                nc.vector.reciprocal_approx_fast(rb_sb[:], pd[:])
                o_sb = []
                for vt in range(2):
                    ot = o_p.tile([128, NBLK], BF16, tag="o",
                                  name=f"o{vt}_{s}")
                    nc.vector.tensor_mul(ot[:], po[vt][:], rb_sb[:])
                    o_sb.append(ot)
                for ot in range(2):
                    pfc = ps_fc.tile([128, NBLK], F32, tag="fc",
                                     name=f"pfc{ot}_{s}")
                    for vt in range(2):
                        nc.tensor.matmul(
                            pfc[:], fcw_t[vt][:, 128 * ot:128 * (ot + 1)],
                            o_sb[vt][:], start=(vt == 0), stop=(vt == 1))
                    t_sb = tr_p.tile([128, NBLK], F32, tag=f"t{ot}",
                                     name=f"t{ot}_{s}")
                    nc.scalar.activation(t_sb[:], pfc[:], Act.Relu,
                                         bias=fcb_t[ot][:])
                    r_sb = tr_p.tile([128, NBLK], F32, tag=f"r{ot}",
                                     name=f"r{ot}_{s}")
                    nc.vector.tensor_add(
                        r_sb[:], t_sb[:],
                        xq_t[ot][:, NBLK * s:NBLK * (s + 1)])
                    nc.sync.dma_start(
                        out_d[128 * ot:128 * (ot + 1),
                              NBLK * s:NBLK * (s + 1)], r_sb[:])

            pending = None  # deferred finalize of previous slot
            for s in range(NSLOT):
                M = M_S[s]
                po = [ps_out.tile([128, NBLK], F32, tag=f"o{vt}",
                                  name=f"po{vt}_{s}") for vt in range(2)]
                pd = ps_den.tile([128, NBLK], F32, tag="den", name=f"pd{s}")
                ex_tiles = [None] * M

                def emit_scores(i, s=s, ex_tiles=ex_tiles, M=M):
                    # scores^T tile [128 keys, 512 queries]
                    psc = ps_sc.tile([128, NBLK], F32, tag="sc",
                                     name=f"psc{s}_{i}")
                    for kt in range(2):
                        nc.tensor.matmul(
                            psc[:],
                            k_sb[kt][i // 4][:, 128 * (i % 4):128 * (i % 4 + 1)],
                            q_sb[kt][s][:], start=(kt == 0), stop=(kt == 1))
                    ex = ex_p.tile([128, NBLK], BF16, tag="ex",
                                   name=f"ex{s}_{i}")
                    nc.scalar.activation(ex[:], psc[:], Act.Exp)
                    if i >= M - 8:
                        mk = msk_p.tile([128, NBLK], BF16, tag="mk",
                                        name=f"mk{s}_{i}")
                        nc.sync.dma_start(mk[:], msk_d[s, i - (M - 8)])
                        nc.vector.tensor_mul(ex[:], ex[:], mk[:])
                    ex_tiles[i] = ex

                def consume_quad(j, po=po, pd=pd, M=M, ex_tiles=ex_tiles,
                                 s=s):
                    for jj in range(j, j + 4):
                        e = ex_tiles[jj]
                        for vt in range(2):
                            nc.tensor.matmul(
                                po[vt][:],
                                vT_sb[jj][:, 128 * vt:128 * (vt + 1)],
                                e[:], start=(jj == 0), stop=(jj == M - 1))
                    # quad-summed denominator: quarters the den matmuls
                    da = ds_p.tile([128, NBLK], BF16, tag="ds",
                                   name=f"da{s}_{j}")
                    nc.vector.tensor_add(da[:], ex_tiles[j][:],
                                         ex_tiles[j + 1][:])
                    db = ds_p.tile([128, NBLK], BF16, tag="ds",
                                   name=f"db{s}_{j}")
                    nc.vector.tensor_add(db[:], ex_tiles[j + 2][:],
                                         ex_tiles[j + 3][:])
                    dsum = ds_p.tile([128, NBLK], BF16, tag="ds",
                                     name=f"ds{s}_{j}")
                    nc.vector.tensor_add(dsum[:], da[:], db[:])
                    nc.tensor.matmul(pd[:], ones_b[:], dsum[:],
                                     start=(j == 0), stop=(j == M - 4))
                    for jj in range(j, j + 4):
                        ex_tiles[jj] = None

                # 4-tile score batches between bf16 consume batches: fewer
                # f32r<->bf16 PE dtype switches (each costs ~100-200ns)
                for ib in range(0, M, 4):
                    for i in range(ib, ib + 4):
                        emit_scores(i)
                    if ib == 4 and pending is not None:
                        finalize_slot(*pending)
                        pending = None
                    if ib >= 4:
                        consume_quad(ib - 4)
                consume_quad(M - 4)
                pending = (s, po, pd)

            finalize_slot(*pending)

    nc.compile()
    return nc


def _host_prep(x, q_w, q_b, k_w, k_b, v_w, v_b, fc_w, fc_b):
    """Build the per-core input maps."""
    import ml_dtypes
    f32 = np.float32
    bf16 = ml_dtypes.bfloat16
    n = np.arange(N)
    px = ((n // S) / S).astype(f32)
    py = ((n % S) / S).astype(f32)
    pos3 = np.stack([px, py, np.ones(N, f32)])   # [3, N] (incl bias channel)

    pos_pad = np.zeros((128, N), f32)
    pos_pad[:3] = pos3

    def pad_w(w, b):
        # rows 0..1 = pos weight rows, row 2 = bias, rest zero
        p = np.zeros((128, K), f32)
        p[:2] = w.astype(f32).T[C:]
        p[2] = b.astype(f32)
        return p

    # per-role masks [NSLOT, 8, 128, 512]
    mm = np.arange(128)[:, None]
    nn = np.arange(NBLK)[None, :]
    masks = {}
    for r in range(2):
        mr = np.zeros((NSLOT, 8, 128, NBLK), f32)
        for s in range(NSLOT):
            j = BLOCKS[r][s]
            for t in range(8):
                i = M_S[s] - 8 + t
                mr[s, t] = (128 * i + mm <= 512 * j + nn)
        masks[r] = mr.astype(bf16)

    shared = {
        "wqf": np.ascontiguousarray(q_w.astype(f32).T[:C]),
        "wkf": np.ascontiguousarray(k_w.astype(f32).T[:C]),
        "wv": np.ascontiguousarray(v_w.astype(f32).T[:C]).astype(bf16),
        "wvc": np.ascontiguousarray(
            np.concatenate([v_w.astype(f32).T[C:],
                            v_b.astype(f32)[None, :]], 0)).astype(bf16),
        "wqp": pad_w(q_w, q_b), "wkp": pad_w(k_w, k_b),
        "x0p": pos_pad,
        "x0c": np.ascontiguousarray(
            np.concatenate([pos3[:2], np.ones((1, N), f32)], 0)).astype(bf16),
        "fcw": np.ascontiguousarray(fc_w.astype(f32).T).astype(bf16),
        "fcb": np.ascontiguousarray(fc_b.astype(f32).reshape(C, 1)),
        "ones_b": np.ones((128, 128), bf16),
    }

    in_maps = []
    for c in range(8):
        b, r = c // 2, c % 2
        xb = x[b].reshape(C, N).astype(f32)
        xq_cols = np.concatenate(
            [np.arange(NBLK * j, NBLK * (j + 1)) for j in BLOCKS[r]])
        in_maps.append(dict(
            shared,
            x0f=np.ascontiguousarray(xb),
            x0b=np.ascontiguousarray(xb).astype(bf16),
            xqf=np.ascontiguousarray(xb[:, xq_cols]),
            xqp=np.ascontiguousarray(pos_pad[:, xq_cols]),
            masks=masks[r],
        ))
    return in_maps


def _gather(results):
    out = np.empty((B, C, N), np.float32)
    for c in range(8):
        b, r = c // 2, c % 2
        oc = results[c]["out"]
        for s, j in enumerate(BLOCKS[r]):
            out[b][:, NBLK * j:NBLK * (j + 1)] = oc[:, NBLK * s:NBLK * (s + 1)]
    return out.reshape(B, C, S, S)


def run(trace=False, **inputs):
    from concourse import bass_utils
    global _PROGRAM
    if _PROGRAM is None:
        _PROGRAM = _build_program()
    in_maps = _host_prep(**inputs)
    res = bass_utils.run_bass_kernel_spmd(
        _PROGRAM, in_maps, list(range(8)), trace=trace)
    return _gather(res.results), res


def kernel(**inputs):
    out, _ = run(trace=False, **inputs)
    return out

